# revision 14
# baseline (speedup 1.0000x reference)
"""Trainium2 Bass kernel for single-query sparse attention with entropy modulation.

Reference computation (per batch b, M=64 timesteps, H=1024):
    Q = last_state @ Wq.T + bq                       [B, H]
    K = all_states @ Wk.T + bk                       [B, M, H]
    V = all_states @ Wv.T + bv                       [B, M, H]
    scores = (Q . K) / sqrt(H) * exp(-We * IE)       [B, M]
    attn = softmax(scores)                           [B, M]
    context = attn @ V                               [B, H]

Algebraic rewrite (softmax attention is linear in K and V):
    Qt = Q @ Wk          -> scores_raw[b,m] = all_states[b,m,:] . Qt[b,:] + Q[b].bk
    context = (sum_m attn[b,m] * all_states[b,m,:]) @ Wv.T + bv   (sum attn = 1)
This removes the two [B*M,H]x[H,H] projections (275 GFLOP) leaving ~7 GFLOP,
making the kernel HBM-bound on streaming all_states once (32 MiB/core).

Sharding: pure data parallel, batch 1024 -> 128 per core across 8 cores.
"""

import os
import numpy as np

import concourse.bacc as bacc
import concourse.bass as bass
import concourse.tile as tile
from concourse import mybir
from concourse.bass_utils import run_bass_kernel_spmd
from concourse.masks import make_identity

F32 = mybir.dt.float32
BF16 = mybir.dt.bfloat16

B, M, H = 1024, 64, 1024
NCORES = 8
BL = B // NCORES          # 128 batch rows per core = SBUF partition dim
NHC = H // 128            # 8 h-chunks of 128
CM = 4                    # timesteps per streamed chunk
NCHUNK = M // CM          # 16 chunks
INV_SQRT_H = 1.0 / 32.0


def _transpose_128(nc, pool_ps, act_dst, src, identity):
    """Transpose a [128, 1024] SBUF tile into act_dst (a [128, 1024] SBUF AP
    holding the 8 transposed 128x128 blocks side by side), via PE + one ACT copy."""
    ps = pool_ps.tile([128, H], F32, tag="tps")
    for j in range(NHC):
        nc.tensor.transpose(ps[:, j * 128:(j + 1) * 128],
                            src[:, j * 128:(j + 1) * 128], identity)
    nc.scalar.copy(out=act_dst, in_=ps[:])


def build_kernel():
    nc = bacc.Bacc("TRN2", target_bir_lowering=False, debug=False)

    last_state = nc.dram_tensor("last_state", [BL, H], F32, kind="ExternalInput")
    all_states = nc.dram_tensor("all_states", [BL, M, H], F32, kind="ExternalInput")
    ies = nc.dram_tensor("IEs", [BL, M], F32, kind="ExternalInput")
    wq = nc.dram_tensor("Wq", [H, H], F32, kind="ExternalInput")
    bq = nc.dram_tensor("bq", [H], F32, kind="ExternalInput")
    wk = nc.dram_tensor("Wk", [H, H], F32, kind="ExternalInput")
    bk = nc.dram_tensor("bk", [H], F32, kind="ExternalInput")
    wv = nc.dram_tensor("Wv", [H, H], F32, kind="ExternalInput")
    bv = nc.dram_tensor("bv", [H], F32, kind="ExternalInput")
    we = nc.dram_tensor("We", [M], F32, kind="ExternalInput")
    context_out = nc.dram_tensor("context", [BL, H], F32, kind="ExternalOutput")
    attn_out = nc.dram_tensor("attn", [BL, M], F32, kind="ExternalOutput")

    with tile.TileContext(nc) as tc:
        with (
            tc.tile_pool(name="const", bufs=1) as const,
            tc.tile_pool(name="wt", bufs=1) as wt_pool,          # WqT then WvT (reused slot)
            tc.tile_pool(name="wk", bufs=NHC) as wk_pool,        # Wk row slabs
            tc.tile_pool(name="wslab", bufs=2) as wslab,         # Wq/Wv row slabs while transposing
            tc.tile_pool(name="mid", bufs=1) as mid,             # X/XT/Q/QT/Qt etc.
            tc.tile_pool(name="astream", bufs=3) as astream,     # streamed all_states chunks
            tc.tile_pool(name="abf", bufs=3) as abf_pool,        # bf16 casts of the chunks
            tc.tile_pool(name="scr", bufs=2) as scr,             # TTR product scratch
            tc.tile_pool(name="diag", bufs=4) as diagp,
            tc.tile_pool(name="ps", bufs=2, space="PSUM") as psp,        # transient [128,1024]
            tc.tile_pool(name="ps_small", bufs=2, space="PSUM") as pss,  # [128,<=64]
            tc.tile_pool(name="ps_ctx", bufs=1, space="PSUM") as psc,    # ctx accumulator
        ):
            # ---------------- constants / small inputs ----------------
            identity = const.tile([128, 128], F32)
            make_identity(nc, identity[:])
            ones_row = const.tile([1, 128], F32)
            nc.vector.memset(ones_row[:], 1.0)

            bq_sb = const.tile([1, H], F32)
            nc.sync.dma_start(out=bq_sb[:], in_=bq[:].unsqueeze(0))
            bv_sb = const.tile([1, H], F32)
            nc.sync.dma_start(out=bv_sb[:], in_=bv[:].unsqueeze(0))
            we_sb = const.tile([1, M], F32)
            nc.sync.dma_start(out=we_sb[:], in_=we[:].unsqueeze(0))
            # bk as [128, 8]: column c holds bk[c*128:(c+1)*128]
            bk_sb = const.tile([128, NHC], F32)
            nc.sync.dma_start(out=bk_sb[:], in_=bk[:].rearrange("(c p) -> p c", p=128))
            ies_sb = const.tile([128, M], F32)
            nc.sync.dma_start(out=ies_sb[:], in_=ies[:])

            x_sb = mid.tile([128, H], F32, tag="x")
            nc.sync.dma_start(out=x_sb[:], in_=last_state[:])

            # ---------------- WqT build (PE transpose, slab pipelined) --------
            # wqt[:, j*1024 + r*128 + k] = Wq[r*128 + k', j*128 + k].T layout:
            # column block j*1024..(j+1)*1024 is WqT[h in chunk j][all i], i.e. the
            # rhs [128, 1024] for the h-chunk j accumulation step of Q = X @ Wq.T.
            wqt = wt_pool.tile([128, NHC * H], F32, tag="wT")
            for rr in range(NHC):
                slab = wslab.tile([128, H], F32, tag="wslab")
                nc.sync.dma_start(out=slab[:], in_=wq[rr * 128:(rr + 1) * 128, :])
                ps = psp.tile([128, H], F32, tag="tps")
                for j in range(NHC):
                    nc.tensor.transpose(ps[:, j * 128:(j + 1) * 128],
                                        slab[:, j * 128:(j + 1) * 128], identity[:])
                dst = wqt[:].rearrange("p (j rb k) -> p j rb k", j=NHC, rb=NHC)[:, :, rr, :]
                nc.scalar.copy(out=dst, in_=ps[:].rearrange("p (j k) -> p j k", j=NHC))

            # Wk needs no transpose: row slab c is the rhs for k-chunk c of Qt = Q @ Wk.
            wk_sb = []
            for c in range(NHC):
                t = wk_pool.tile([128, H], F32, tag="wk")
                nc.sync.dma_start(out=t[:], in_=wk[c * 128:(c + 1) * 128, :])
                wk_sb.append(t)

            # ---------------- X.T ----------------
            xt_sb = mid.tile([128, H], F32, tag="xt")
            _transpose_128(nc, psp, xt_sb[:], x_sb[:], identity[:])

            # ---------------- Q = X @ Wq.T + bq  (psum [128 b, 1024 i]) -------
            q_ps = psp.tile([128, H], F32, tag="tps")
            for c in range(NHC):
                lhsT = xt_sb[:, c * 128:(c + 1) * 128]
                rhs = wqt[:, c * H:(c + 1) * H]
                nc.tensor.matmul(q_ps[:, 0:512], lhsT, rhs[:, 0:512],
                                 start=(c == 0), stop=False)
                nc.tensor.matmul(q_ps[:, 512:H], lhsT, rhs[:, 512:H],
                                 start=(c == 0), stop=False)
            nc.tensor.matmul(q_ps[:, 0:512], ones_row[:], bq_sb[:, 0:512],
                             start=False, stop=True)
            nc.tensor.matmul(q_ps[:, 512:H], ones_row[:], bq_sb[:, 512:H],
                             start=False, stop=True)
            q_sb = mid.tile([128, H], F32, tag="q")
            nc.scalar.copy(out=q_sb[:], in_=q_ps[:])

            # ---------------- Q.T ----------------
            qt_t_sb = mid.tile([128, H], F32, tag="qtt")
            _transpose_128(nc, psp, qt_t_sb[:], q_sb[:], identity[:])

            # ---------------- Qt = Q @ Wk  (psum [128 b, 1024 h']) ------------
            qt_ps = psp.tile([128, H], F32, tag="tps")
            for c in range(NHC):
                lhsT = qt_t_sb[:, c * 128:(c + 1) * 128]
                nc.tensor.matmul(qt_ps[:, 0:512], lhsT, wk_sb[c][:, 0:512],
                                 start=(c == 0), stop=(c == NHC - 1))
                nc.tensor.matmul(qt_ps[:, 512:H], lhsT, wk_sb[c][:, 512:H],
                                 start=(c == 0), stop=(c == NHC - 1))
            qt_sb = mid.tile([128, H], F32, tag="qt")
            nc.scalar.copy(out=qt_sb[:], in_=qt_ps[:])

            # ---------------- qb = Q . bk  (psum [128 b, 1]) ------------------
            qb_ps = pss.tile([128, 1], F32, tag="ps_small")
            for c in range(NHC):
                nc.tensor.matmul(qb_ps[:], qt_t_sb[:, c * 128:(c + 1) * 128],
                                 bk_sb[:, c:c + 1],
                                 start=(c == 0), stop=(c == NHC - 1))
            qb_col = mid.tile([128, 1], F32, tag="qb")
            nc.scalar.copy(out=qb_col[:], in_=qb_ps[:])

            # ---------------- modulation Mt = exp(-We*IE)/sqrt(H) -------------
            we_ps = pss.tile([128, M], F32, tag="ps_small")
            nc.tensor.matmul(we_ps[:], ones_row[:], we_sb[:], start=True, stop=True)
            mt_pre = mid.tile([128, M], F32, tag="mtp")
            nc.vector.tensor_mul(out=mt_pre[:], in0=ies_sb[:], in1=we_ps[:])
            mt_sb = mid.tile([128, M], F32, tag="mt")
            lninv = const.tile([128, 1], F32)
            nc.vector.memset(lninv[:], float(np.log(INV_SQRT_H)))
            zero_col = const.tile([128, 1], F32)
            nc.vector.memset(zero_col[:], 0.0)
            nc.scalar.activation(out=mt_sb[:], in_=mt_pre[:],
                                 func=mybir.ActivationFunctionType.Exp,
                                 scale=-1.0, bias=lninv[:])

            # ---------------- streaming state ----------------
            s_raw = mid.tile([128, M], F32, tag="sraw")
            s_mod = mid.tile([128, M], F32, tag="smod")
            p_sb = mid.tile([128, M], F32, tag="p")
            dparts = mid.tile([128, NCHUNK], F32, tag="dparts")
            c_ps = psc.tile([128, H], F32, tag="C")

            def emit_ctx_mms(g, a_bf):
                """diag(p_m) matmuls accumulating p_m * A_m into c_ps (bf16 inputs,
                fp32 PSUM accumulation; GPSIMD builds the diags so the DVE never
                needs the shared SBUF port during streaming)."""
                for mm in range(CM):
                    m = g * CM + mm
                    dg = diagp.tile([128, 128], BF16, tag="diag")
                    nc.gpsimd.tensor_scalar_mul(out=dg[:], in0=identity[:],
                                                scalar1=p_sb[:, m:m + 1])
                    nc.tensor.matmul(c_ps[:, 0:512], dg[:], a_bf[:, mm, 0:512],
                                     start=(m == 0), stop=(m == M - 1),
                                     skip_group_check=True)
                    nc.tensor.matmul(c_ps[:, 512:H], dg[:], a_bf[:, mm, 512:H],
                                     start=(m == 0), stop=(m == M - 1),
                                     skip_group_check=True)

            # ---------------- stream all_states ----------------
            prev = None  # (g, a_t) whose exp() is in flight
            for g in range(NCHUNK):
                a_t = astream.tile([128, CM, H], F32, tag="A")
                nc.sync.dma_start(out=a_t[:], in_=all_states[:, g * CM:(g + 1) * CM, :])
                a_bf = abf_pool.tile([128, CM, H], BF16, tag="Abf")
                nc.gpsimd.tensor_copy(out=a_bf[:], in_=a_t[:])
                for mm in range(CM):
                    m = g * CM + mm
                    scratch = scr.tile([128, H], F32, tag="ttr")
                    nc.vector.tensor_mul(out=scratch[:], in0=a_t[:, mm, :], in1=qt_sb[:])
                    # free-dim reduction on ScalarE (in-place Copy + accum_out)
                    nc.scalar.activation(out=scratch[:], in_=scratch[:],
                                         func=mybir.ActivationFunctionType.Copy,
                                         accum_out=s_raw[:, m:m + 1])
                sl = slice(g * CM, (g + 1) * CM)
                nc.vector.tensor_scalar_add(out=s_mod[:, sl], in0=s_raw[:, sl],
                                            scalar1=qb_col[:])
                nc.vector.tensor_mul(out=s_mod[:, sl], in0=s_mod[:, sl], in1=mt_sb[:, sl])
                nc.scalar.activation(out=p_sb[:, sl], in_=s_mod[:, sl],
                                     func=mybir.ActivationFunctionType.Exp,
                                     bias=zero_col[:],
                                     accum_out=dparts[:, g:g + 1])
                if prev is not None:
                    emit_ctx_mms(*prev)
                prev = (g, a_bf)
            emit_ctx_mms(*prev)

            # ---------------- WvT build (emitted last; scheduler runs it when
            # the wt slot frees after the Q GEMM and DMA bandwidth allows) -----
            wvt = wt_pool.tile([128, NHC * H], F32, tag="wT")
            for rr in range(NHC):
                slab = wslab.tile([128, H], F32, tag="wslab")
                nc.sync.dma_start(out=slab[:], in_=wv[rr * 128:(rr + 1) * 128, :])
                ps = psp.tile([128, H], F32, tag="tps")
                for j in range(NHC):
                    nc.tensor.transpose(ps[:, j * 128:(j + 1) * 128],
                                        slab[:, j * 128:(j + 1) * 128], identity[:])
                dst = wvt[:].rearrange("p (j rb k) -> p j rb k", j=NHC, rb=NHC)[:, :, rr, :]
                nc.scalar.copy(out=dst, in_=ps[:].rearrange("p (j k) -> p j k", j=NHC))

            # ---------------- softmax denominator + attn out ------------------
            denom = mid.tile([128, 1], F32, tag="denom")
            nc.vector.reduce_sum(out=denom[:], in_=dparts[:], axis=mybir.AxisListType.X)
            recip = mid.tile([128, 1], F32, tag="recip")
            nc.vector.reciprocal(out=recip[:], in_=denom[:])
            attn_sb = mid.tile([128, M], F32, tag="attn")
            nc.vector.tensor_scalar_mul(out=attn_sb[:], in0=p_sb[:], scalar1=recip[:])
            nc.sync.dma_start(out=attn_out[:], in_=attn_sb[:])

            # ---------------- ctx_pre = C / denom, transpose ------------------
            ctx_sb = mid.tile([128, H], F32, tag="ctx")
            nc.vector.tensor_scalar_mul(out=ctx_sb[:], in0=c_ps[:], scalar1=recip[:])
            ctxt_sb = mid.tile([128, H], F32, tag="ctxt")
            _transpose_128(nc, psp, ctxt_sb[:], ctx_sb[:], identity[:])

            # ---------------- context = ctx_pre @ Wv.T + bv -------------------
            o_ps = psp.tile([128, H], F32, tag="tps")
            for c in range(NHC):
                lhsT = ctxt_sb[:, c * 128:(c + 1) * 128]
                rhs = wvt[:, c * H:(c + 1) * H]
                nc.tensor.matmul(o_ps[:, 0:512], lhsT, rhs[:, 0:512],
                                 start=(c == 0), stop=False)
                nc.tensor.matmul(o_ps[:, 512:H], lhsT, rhs[:, 512:H],
                                 start=(c == 0), stop=False)
            nc.tensor.matmul(o_ps[:, 0:512], ones_row[:], bv_sb[:, 0:512],
                             start=False, stop=True)
            nc.tensor.matmul(o_ps[:, 512:H], ones_row[:], bv_sb[:, 512:H],
                             start=False, stop=True)
            out_sb = mid.tile([128, H], F32, tag="out")
            nc.scalar.copy(out=out_sb[:], in_=o_ps[:])
            nc.sync.dma_start(out=context_out[:], in_=out_sb[:])

    nc.compile()
    return nc


_NC_CACHE = None


def _get_nc():
    global _NC_CACHE
    if _NC_CACHE is None:
        _NC_CACHE = build_kernel()
    return _NC_CACHE


def kernel(**inputs):
    nc = _get_nc()
    arr = {k: np.asarray(v, dtype=np.float32) for k, v in inputs.items()}
    in_maps = []
    for c in range(NCORES):
        sl = slice(c * BL, (c + 1) * BL)
        in_maps.append({
            "last_state": arr["last_state"][sl],
            "all_states": arr["all_states"][sl],
            "IEs": np.ascontiguousarray(arr["IEs"][sl, :, 0]),
            "Wq": arr["Wq"], "bq": arr["bq"],
            "Wk": arr["Wk"], "bk": arr["bk"],
            "Wv": arr["Wv"], "bv": arr["bv"],
            "We": arr["We"],
        })
    trace = bool(int(os.environ.get("KERNEL_TRACE", "0")))
    res = run_bass_kernel_spmd(nc, in_maps, core_ids=list(range(NCORES)),
                               trace=trace)
    if trace:
        kernel.last_result = res
    context = np.concatenate([res.results[c]["context"] for c in range(NCORES)], axis=0)
    attn = np.concatenate([res.results[c]["attn"] for c in range(NCORES)], axis=0)
    return context, attn


if __name__ == "__main__":
    rng = np.random.default_rng(0)
    fake = {
        "last_state": rng.standard_normal((B, H), dtype=np.float32),
        "all_states": rng.standard_normal((B, M, H), dtype=np.float32),
        "IEs": rng.random((B, M, 1), dtype=np.float32),
        "Wq": rng.standard_normal((H, H), dtype=np.float32) / 32,
        "bq": rng.standard_normal((H,), dtype=np.float32) / 32,
        "Wk": rng.standard_normal((H, H), dtype=np.float32) / 32,
        "bk": rng.standard_normal((H,), dtype=np.float32) / 32,
        "Wv": rng.standard_normal((H, H), dtype=np.float32) / 32,
        "bv": rng.standard_normal((H,), dtype=np.float32) / 32,
        "We": 1.0 + 0.01 * rng.standard_normal((M,), dtype=np.float32),
    }
    ctx, at = kernel(**fake)
    print("kernel ran:", ctx.shape, at.shape)


# revision 17
# speedup vs baseline: 77.3844x; 77.3844x over previous
"""Trainium2 Bass kernel for single-query sparse attention with entropy modulation.

Reference computation (per batch b, M=64 timesteps, H=1024):
    Q = last_state @ Wq.T + bq                       [B, H]
    K = all_states @ Wk.T + bk                       [B, M, H]
    V = all_states @ Wv.T + bv                       [B, M, H]
    scores = (Q . K) / sqrt(H) * exp(-We * IE)       [B, M]
    attn = softmax(scores)                           [B, M]
    context = attn @ V                               [B, H]

Algebraic rewrite (softmax attention is linear in K and V):
    Qt = Q @ Wk     ->  scores_raw[b,m] = all_states[b,m,:] . Qt[b,:] + Q[b].bk
    context = (sum_m attn[b,m] * all_states[b,m,:]) @ Wv.T + bv    (sum attn = 1)
This removes the two [B*M,H]x[H,H] projections (275 GFLOP) leaving ~7 GFLOP,
making the kernel HBM-bound on streaming all_states once (32 MiB/core).

Implementation notes:
  - batch is data-parallel across the 8 cores (128 rows/core = partition dim)
  - all_states streams in as bf16 via SWDGE cast-DMA (HBM reads stay fp32-sized)
  - per-m dot products: one DVE multiply per chunk + free-dim reductions split
    between ScalarE (activation accum_out) and the DVE (tensor_reduce)
  - the attention-weighted sum over m runs on the PE as diag(p_m) matmuls
    accumulating natively in PSUM; diag masks are built by GPSIMD so the DVE
    never touches the shared SBUF port mid-stream
  - projections and transposes run on the PE in bf16; fp32 accumulation in PSUM
"""

import os
import numpy as np

import concourse.bacc as bacc
import concourse.bass as bass
import concourse.tile as tile
from concourse import mybir
from concourse.bass_utils import run_bass_kernel_spmd
from concourse.masks import make_identity

F32 = mybir.dt.float32
BF16 = mybir.dt.bfloat16

B, M, H = 1024, 64, 1024
NCORES = 8
BL = B // NCORES          # 128 batch rows per core = SBUF partition dim
NHC = H // 128            # 8 h-chunks of 128
CM = 4                    # timesteps per streamed chunk
NCHUNK = M // CM          # 16 chunks
INV_SQRT_H = 1.0 / 32.0


def build_kernel(repeat=1):
    nc = bacc.Bacc("TRN2", target_bir_lowering=False, debug=False)

    last_state = nc.dram_tensor("last_state", [BL, H], F32, kind="ExternalInput")
    all_states = nc.dram_tensor("all_states", [BL, M, H], F32, kind="ExternalInput")
    ies = nc.dram_tensor("IEs", [BL, M], F32, kind="ExternalInput")
    wq = nc.dram_tensor("Wq", [H, H], F32, kind="ExternalInput")
    bq = nc.dram_tensor("bq", [H], F32, kind="ExternalInput")
    wk = nc.dram_tensor("Wk", [H, H], F32, kind="ExternalInput")
    bk = nc.dram_tensor("bk", [H], F32, kind="ExternalInput")
    wv = nc.dram_tensor("Wv", [H, H], F32, kind="ExternalInput")
    bv = nc.dram_tensor("bv", [H], F32, kind="ExternalInput")
    we = nc.dram_tensor("We", [M], F32, kind="ExternalInput")
    context_out = nc.dram_tensor("context", [BL, H], F32, kind="ExternalOutput")
    attn_out = nc.dram_tensor("attn", [BL, M], F32, kind="ExternalOutput")

    with tile.TileContext(nc) as tc:
      for _rep in range(repeat):
        with (
            tc.tile_pool(name="const", bufs=1) as const,
            tc.tile_pool(name="wt", bufs=1) as wt_pool,
            tc.tile_pool(name="wkp", bufs=NHC) as wk_pool,
            tc.tile_pool(name="wslab", bufs=2) as wslab,
            tc.tile_pool(name="mid", bufs=1) as mid,
            tc.tile_pool(name="abf", bufs=6) as abf_pool,
            tc.tile_pool(name="scr", bufs=3) as scr,
            tc.tile_pool(name="diag", bufs=4) as diagp,
            tc.tile_pool(name="ps32", bufs=1, space="PSUM") as ps32,     # GEMM accum
            tc.tile_pool(name="psbf", bufs=2, space="PSUM") as psbf,     # bf16 transposes
            tc.tile_pool(name="ps_small", bufs=2, space="PSUM") as pss,
            tc.tile_pool(name="ps_ctx", bufs=1, space="PSUM") as psc,
        ):
            # ---------------- constants / small inputs ----------------
            identity = const.tile([128, 128], BF16)
            make_identity(nc, identity[:])
            ones_bf = const.tile([1, 128], BF16)
            nc.vector.memset(ones_bf[:], 1.0)

            bq_bf = const.tile([1, H], BF16)
            nc.gpsimd.dma_start(out=bq_bf[:], in_=bq[:].unsqueeze(0))
            bv_bf = const.tile([1, H], BF16)
            nc.gpsimd.dma_start(out=bv_bf[:], in_=bv[:].unsqueeze(0))
            we_sb = const.tile([1, M], F32)
            nc.sync.dma_start(out=we_sb[:], in_=we[:].unsqueeze(0))
            # bk as [128, 8] bf16: column c holds bk[c*128:(c+1)*128]
            bk_bf = const.tile([128, NHC], BF16)
            nc.gpsimd.dma_start(out=bk_bf[:], in_=bk[:].rearrange("(c p) -> p c", p=128))
            ies_sb = const.tile([128, M], F32)
            nc.sync.dma_start(out=ies_sb[:], in_=ies[:])
            x_bf = mid.tile([128, H], BF16, tag="x")
            nc.gpsimd.dma_start(out=x_bf[:], in_=last_state[:])
            lninv = const.tile([128, 1], F32)
            nc.vector.memset(lninv[:], float(np.log(INV_SQRT_H)))
            zero_col = const.tile([128, 1], F32)
            nc.vector.memset(zero_col[:], 0.0)

            def transpose_1024(src_bf, dst_bf, on_vector=True):
                """src [128,1024] bf16 -> dst bf16 with the 8 transposed 128x128
                blocks side by side (PE transpose + one PSUM->SBUF copy)."""
                ps = psbf.tile([128, H], BF16, tag="tpsbf")
                for j in range(NHC):
                    nc.tensor.transpose(ps[:, j * 128:(j + 1) * 128],
                                        src_bf[:, j * 128:(j + 1) * 128], identity[:])
                if on_vector:
                    nc.vector.tensor_copy(out=dst_bf, in_=ps[:])
                else:
                    nc.scalar.copy(out=dst_bf, in_=ps[:])

            # ---------------- WqT build (bf16 slabs -> PE transpose) ----------
            # wT column block j*1024..(j+1)*1024 = rhs [128 h, 1024 i] for the
            # h-chunk j accumulation step (cols ordered j*1024 + r*128 + k).
            def build_wT(w_dram, dst, copy_parity):
                for rr in range(NHC):
                    slab = wslab.tile([128, H], BF16, tag="wslab")
                    nc.gpsimd.dma_start(out=slab[:],
                                        in_=w_dram[rr * 128:(rr + 1) * 128, :])
                    ps = psbf.tile([128, H], BF16, tag="tpsbf")
                    for j in range(NHC):
                        nc.tensor.transpose(ps[:, j * 128:(j + 1) * 128],
                                            slab[:, j * 128:(j + 1) * 128], identity[:])
                    dst_ap = dst[:].rearrange("p (j rb k) -> p j rb k",
                                              j=NHC, rb=NHC)[:, :, rr, :]
                    src_ap = ps[:].rearrange("p (j k) -> p j k", j=NHC)
                    if rr % 2 == copy_parity:
                        nc.scalar.copy(out=dst_ap, in_=src_ap)
                    else:
                        nc.vector.tensor_copy(out=dst_ap, in_=src_ap)

            wqt = wt_pool.tile([128, NHC * H], BF16, tag="wqt")
            build_wT(wq, wqt, 0)

            wk_sb = []
            for c in range(NHC):
                t = wk_pool.tile([128, H], BF16, tag="wk")
                nc.gpsimd.dma_start(out=t[:], in_=wk[c * 128:(c + 1) * 128, :])
                wk_sb.append(t)

            # ---------------- X.T ----------------
            xt_bf = mid.tile([128, H], BF16, tag="xt")
            transpose_1024(x_bf[:], xt_bf[:])

            # ---------------- Q = X @ Wq.T + bq  (psum [128 b, 1024 i]) -------
            q_ps = ps32.tile([128, H], F32, tag="gemm")
            for c in range(NHC):
                lhsT = xt_bf[:, c * 128:(c + 1) * 128]
                rhs = wqt[:, c * H:(c + 1) * H]
                nc.tensor.matmul(q_ps[:, 0:512], lhsT, rhs[:, 0:512],
                                 start=(c == 0), stop=False)
                nc.tensor.matmul(q_ps[:, 512:H], lhsT, rhs[:, 512:H],
                                 start=(c == 0), stop=False)
            nc.tensor.matmul(q_ps[:, 0:512], ones_bf[:], bq_bf[:, 0:512],
                             start=False, stop=True)
            nc.tensor.matmul(q_ps[:, 512:H], ones_bf[:], bq_bf[:, 512:H],
                             start=False, stop=True)
            q_bf = mid.tile([128, H], BF16, tag="q")
            nc.scalar.copy(out=q_bf[:], in_=q_ps[:])

            # ---------------- Q.T ----------------
            qt_t_bf = mid.tile([128, H], BF16, tag="qtt")
            transpose_1024(q_bf[:], qt_t_bf[:])

            # ---------------- Qt = Q @ Wk  (psum [128 b, 1024 h']) ------------
            qt_ps = ps32.tile([128, H], F32, tag="gemm")
            for c in range(NHC):
                lhsT = qt_t_bf[:, c * 128:(c + 1) * 128]
                nc.tensor.matmul(qt_ps[:, 0:512], lhsT, wk_sb[c][:, 0:512],
                                 start=(c == 0), stop=(c == NHC - 1))
                nc.tensor.matmul(qt_ps[:, 512:H], lhsT, wk_sb[c][:, 512:H],
                                 start=(c == 0), stop=(c == NHC - 1))
            qt_bf = mid.tile([128, H], BF16, tag="qt")
            nc.scalar.copy(out=qt_bf[:], in_=qt_ps[:])

            # ---------------- qb = Q . bk  (psum [128 b, 1]) ------------------
            qb_ps = pss.tile([128, 1], F32, tag="ps_small")
            for c in range(NHC):
                nc.tensor.matmul(qb_ps[:], qt_t_bf[:, c * 128:(c + 1) * 128],
                                 bk_bf[:, c:c + 1],
                                 start=(c == 0), stop=(c == NHC - 1))
            qb_col = mid.tile([128, 1], F32, tag="qb")
            nc.scalar.copy(out=qb_col[:], in_=qb_ps[:])

            # ---------------- modulation Mt = exp(-We*IE)/sqrt(H) -------------
            ones_f32 = const.tile([1, 128], F32)
            nc.vector.memset(ones_f32[:], 1.0)
            we_ps = pss.tile([128, M], F32, tag="ps_small")
            nc.tensor.matmul(we_ps[:], ones_f32[:], we_sb[:], start=True, stop=True)
            mt_pre = mid.tile([128, M], F32, tag="mtp")
            nc.vector.tensor_mul(out=mt_pre[:], in0=ies_sb[:], in1=we_ps[:])
            mt_sb = mid.tile([128, M], F32, tag="mt")
            nc.scalar.activation(out=mt_sb[:], in_=mt_pre[:],
                                 func=mybir.ActivationFunctionType.Exp,
                                 scale=-1.0, bias=lninv[:])

            # ---------------- streaming state ----------------
            s_raw = mid.tile([128, M], F32, tag="sraw")
            s_mod = mid.tile([128, M], F32, tag="smod")
            p_sb = mid.tile([128, M], F32, tag="p")
            dparts = mid.tile([128, NCHUNK], F32, tag="dparts")
            c_ps = psc.tile([128, H], F32, tag="C")

            def emit_ctx_mms(g, a_bf):
                """diag(p_m) matmuls accumulating p_m * A_m into c_ps (bf16 in,
                fp32 PSUM accumulation; GPSIMD builds the diag masks so the DVE
                never needs the shared SBUF port mid-stream)."""
                for mm in range(CM):
                    m = g * CM + mm
                    dg = diagp.tile([128, 128], BF16, tag="diag")
                    nc.gpsimd.tensor_scalar_mul(out=dg[:], in0=identity[:],
                                                scalar1=p_sb[:, m:m + 1])
                    nc.tensor.matmul(c_ps[:, 0:512], dg[:], a_bf[:, mm, 0:512],
                                     start=(m == 0), stop=(m == M - 1),
                                     skip_group_check=True)
                    nc.tensor.matmul(c_ps[:, 512:H], dg[:], a_bf[:, mm, 512:H],
                                     start=(m == 0), stop=(m == M - 1),
                                     skip_group_check=True)

            # ---------------- stream all_states (bf16 cast-DMA) ---------------
            prev = None
            for g in range(NCHUNK):
                a_bf = abf_pool.tile([128, CM, H], BF16, tag="Abf")
                nc.gpsimd.dma_start(out=a_bf[:],
                                    in_=all_states[:, g * CM:(g + 1) * CM, :])
                prod = scr.tile([128, CM, H], BF16, tag="prod")
                nc.vector.tensor_mul(
                    out=prod[:], in0=a_bf[:],
                    in1=qt_bf[:].unsqueeze(1).broadcast_to((128, CM, H)))
                if g % 2 == 0:
                    # free-dim reduction on ScalarE (Copy + accum_out), per m
                    for mm in range(CM):
                        m = g * CM + mm
                        nc.scalar.activation(out=prod[:, mm, :], in_=prod[:, mm, :],
                                             func=mybir.ActivationFunctionType.Copy,
                                             accum_out=s_raw[:, m:m + 1])
                else:
                    nc.vector.reduce_sum(out=s_raw[:, g * CM:(g + 1) * CM],
                                         in_=prod[:], axis=mybir.AxisListType.X)
                sl = slice(g * CM, (g + 1) * CM)
                nc.vector.tensor_scalar_add(out=s_mod[:, sl], in0=s_raw[:, sl],
                                            scalar1=qb_col[:])
                nc.vector.tensor_mul(out=s_mod[:, sl], in0=s_mod[:, sl],
                                     in1=mt_sb[:, sl])
                nc.scalar.activation(out=p_sb[:, sl], in_=s_mod[:, sl],
                                     func=mybir.ActivationFunctionType.Exp,
                                     bias=zero_col[:],
                                     accum_out=dparts[:, g:g + 1])
                if prev is not None:
                    emit_ctx_mms(*prev)
                prev = (g, a_bf)
            emit_ctx_mms(*prev)

            # ---------------- WvT build (late; slot frees after Q GEMM) -------
            wvt = wt_pool.tile([128, NHC * H], BF16, tag="wvt")
            build_wT(wv, wvt, 1)

            # ---------------- softmax denominator + attn out ------------------
            denom = mid.tile([128, 1], F32, tag="denom")
            nc.vector.reduce_sum(out=denom[:], in_=dparts[:],
                                 axis=mybir.AxisListType.X)
            recip = mid.tile([128, 1], F32, tag="recip")
            nc.vector.reciprocal(out=recip[:], in_=denom[:])
            attn_sb = mid.tile([128, M], F32, tag="attn")
            nc.vector.tensor_scalar_mul(out=attn_sb[:], in0=p_sb[:], scalar1=recip[:])
            nc.sync.dma_start(out=attn_out[:], in_=attn_sb[:])

            # ---------------- ctx_pre = C / denom, transpose ------------------
            ctx_bf = mid.tile([128, H], BF16, tag="ctx")
            nc.vector.tensor_scalar_mul(out=ctx_bf[:], in0=c_ps[:], scalar1=recip[:])
            ctxt_bf = mid.tile([128, H], BF16, tag="ctxt")
            transpose_1024(ctx_bf[:], ctxt_bf[:])

            # ---------------- context = ctx_pre @ Wv.T + bv -------------------
            o_ps = ps32.tile([128, H], F32, tag="gemm")
            for c in range(NHC):
                lhsT = ctxt_bf[:, c * 128:(c + 1) * 128]
                rhs = wvt[:, c * H:(c + 1) * H]
                nc.tensor.matmul(o_ps[:, 0:512], lhsT, rhs[:, 0:512],
                                 start=(c == 0), stop=False)
                nc.tensor.matmul(o_ps[:, 512:H], lhsT, rhs[:, 512:H],
                                 start=(c == 0), stop=False)
            nc.tensor.matmul(o_ps[:, 0:512], ones_bf[:], bv_bf[:, 0:512],
                             start=False, stop=True)
            nc.tensor.matmul(o_ps[:, 512:H], ones_bf[:], bv_bf[:, 512:H],
                             start=False, stop=True)
            out_sb = mid.tile([128, H], F32, tag="out")
            nc.scalar.copy(out=out_sb[:], in_=o_ps[:])
            nc.sync.dma_start(out=context_out[:], in_=out_sb[:])

    nc.compile()
    return nc


_NC_CACHE = None


def _get_nc():
    global _NC_CACHE
    if _NC_CACHE is None:
        _NC_CACHE = build_kernel()
    return _NC_CACHE


def kernel(**inputs):
    nc = _get_nc()
    arr = {k: np.asarray(v, dtype=np.float32) for k, v in inputs.items()}
    in_maps = []
    for c in range(NCORES):
        sl = slice(c * BL, (c + 1) * BL)
        in_maps.append({
            "last_state": arr["last_state"][sl],
            "all_states": arr["all_states"][sl],
            "IEs": np.ascontiguousarray(arr["IEs"][sl, :, 0]),
            "Wq": arr["Wq"], "bq": arr["bq"],
            "Wk": arr["Wk"], "bk": arr["bk"],
            "Wv": arr["Wv"], "bv": arr["bv"],
            "We": arr["We"],
        })
    trace = bool(int(os.environ.get("KERNEL_TRACE", "0")))
    res = run_bass_kernel_spmd(nc, in_maps, core_ids=list(range(NCORES)),
                               trace=trace)
    if trace:
        kernel.last_result = res
    context = np.concatenate([res.results[c]["context"] for c in range(NCORES)],
                             axis=0)
    attn = np.concatenate([res.results[c]["attn"] for c in range(NCORES)], axis=0)
    return context, attn


if __name__ == "__main__":
    rng = np.random.default_rng(0)
    fake = {
        "last_state": rng.standard_normal((B, H), dtype=np.float32),
        "all_states": rng.standard_normal((B, M, H), dtype=np.float32),
        "IEs": rng.random((B, M, 1), dtype=np.float32),
        "Wq": rng.standard_normal((H, H), dtype=np.float32) / 32,
        "bq": rng.standard_normal((H,), dtype=np.float32) / 32,
        "Wk": rng.standard_normal((H, H), dtype=np.float32) / 32,
        "bk": rng.standard_normal((H,), dtype=np.float32) / 32,
        "Wv": rng.standard_normal((H, H), dtype=np.float32) / 32,
        "bv": rng.standard_normal((H,), dtype=np.float32) / 32,
        "We": (1.0 + 0.01 * rng.standard_normal((M,))).astype(np.float32),
    }
    ctx, at = kernel(**fake)
    print("kernel ran:", ctx.shape, at.shape)
